# revision 22
# baseline (speedup 1.0000x reference)
"""Bidirectional cross-attention kernel for Trainium2 (8 NeuronCores).

Full inputs in, full outputs out. Sharding: data-parallel over batch
(B=8 -> one batch element per core), so no collectives are needed.

Per-core computation (S=2048, D=E=1024):
  Q = q @ Wq + bq ; K = k @ Wk + bk ; V = v @ Wv + bv
  out = softmax(Q K^T / 32) @ V

Key algebraic folding (exact): softmax over k is invariant to per-q
constants, so with M = Wk @ Wq^T,
  scoresT[k, q] = (k @ M @ q^T)[k, q]/32 + t3[k]/32 (+ per-q terms that
  cancel), where t3 = k @ (Wk @ bq).
M is folded on the HOST, so the device runs ONE projection
(TT = M-proj of k) instead of separate Q and K projections, and the raw
transposed q input feeds the score matmuls directly. t3 rides along as
a free per-partition bias on the exp activation.

Layout strategy (contraction dim always on partitions, all matmuls
fp16 at N=512 with fp32 PSUM accumulation):
  - host passes qT/kT/vT [D, S] fp16: projections need no transpose
  - TT [D, S] and V [S, E] stay SBUF-resident fp16; qT streams per strip
  - scoresT [k, q] orientation means exp(scoresT) is directly the lhsT
    of the attn @ V matmul -- no transposes anywhere
  - softmax skips max-subtraction (scores ~ N(0,1) after 1/32 scale);
    row sums via ones-vector matmul of a DVE-accumulated attnT tile,
    applied as a reciprocal multiply on the output psum
  - output stored fp16 (host upcasts to f32): halves store traffic/tail

Timing model (measured): exec ~= preamble (6.6us, runtime-fixed) +
HAM power ramp to first full-speed matmul (~12us, covered by warmup
matmuls; idle gaps reset the ramp) + matmul spine (1536 x 213.3ns +
16 rowsums ~= 332us, at 100% of the 2.4GHz fp16 PE rate) + tail
(~4.5us: last store's DMA pipeline latency + Tile end barrier).
fp8/DoubleRow would halve the spine but e4m3's ~2.6% RMS quantization
noise per matmul operand puts end-to-end rel err at 3.7-7.5% (numpy
simulation), far over the 2e-2 budget; hi/lo splitting restores
accuracy but costs 2x matmuls, negating the 1.44x DoubleRow gain.
"""

import numpy as np

P = 128
S = 2048
D = 1024
E = 1024
DC = D // P  # contraction chunks (8)
EB = E // P  # output blocks for TT (8)
SB = S // P  # s blocks for V / k blocks (16)
PSTRIP = 512  # phase-1 free-dim strip
NPS = S // PSTRIP  # 4
QSTRIP = 512  # phase-2 q strip
NQS = S // QSTRIP  # 4
NQB = QSTRIP // P  # 4
ESTRIP = 512
NES = E // ESTRIP  # 2
SCALE = 1.0 / 32.0  # 1/sqrt(E)
NWARM = 88  # PE warmup matmuls: HAM needs ~4-5.4us of CONTINUOUS PE activity
# to reach K=8/8 and an idle gap >~2us resets the ramp (catastrophic: +4us),
# so warmups (~55-80ns each) must bridge past worst-case first data arrival
# (~13.5us; the hardware-dynamic DMA queue pipeline takes ~5us to deliver
# the first chunks no matter which engine issues them).

_CACHE = {}


def _build():
    import concourse.mybir as mybir
    from concourse import bacc
    from concourse.tile import TileContext

    f32 = mybir.dt.float32
    f16 = mybir.dt.float16
    AF = mybir.ActivationFunctionType

    nc = bacc.Bacc()

    qt = nc.dram_tensor("qt", (D, S), f16, kind="ExternalInput")
    kt = nc.dram_tensor("kt", (D, S), f16, kind="ExternalInput")
    vt = nc.dram_tensor("vt", (D, S), f16, kind="ExternalInput")
    wm = nc.dram_tensor("wm", (D, D), f16, kind="ExternalInput")
    wv = nc.dram_tensor("wv", (D, E), f16, kind="ExternalInput")
    t3s = nc.dram_tensor("t3s", (P, SB), f32, kind="ExternalInput")
    bvb = nc.dram_tensor("bvb", (P, E), f32, kind="ExternalInput")
    # fp16 output (host upcasts): halves the store traffic + drain tail
    out = nc.dram_tensor("out", (S, E), f16, kind="ExternalOutput")

    qt_v = qt[:].rearrange("(o p) s -> p o s", p=P)
    kt_v = kt[:].rearrange("(o p) s -> p o s", p=P)
    vt_v = vt[:].rearrange("(o p) s -> p o s", p=P)
    wm_v = wm[:].rearrange("(o p) e -> p o e", p=P)
    wv_v = wv[:].rearrange("(o p) e -> p o e", p=P)

    with TileContext(nc) as tc:
        with (
            tc.tile_pool(name="w", bufs=9) as pool_w,
            tc.tile_pool(name="inp", bufs=3) as pool_in,
            tc.tile_pool(name="wr", bufs=1) as pool_wr,
            tc.tile_pool(name="in0", bufs=8) as pool_in0,
            tc.tile_pool(name="res", bufs=1) as pool_res,
            tc.tile_pool(name="stage", bufs=3) as pool_stage,
            tc.tile_pool(name="at", bufs=18) as pool_at,
            tc.tile_pool(name="acc", bufs=2) as pool_acc,
            tc.tile_pool(name="small", bufs=4) as pool_small,
            tc.tile_pool(name="const", bufs=1) as pool_const,
            tc.tile_pool(name="po", bufs=7, space="PSUM") as pool_po,
            tc.tile_pool(name="pr", bufs=1, space="PSUM") as pool_pr,
        ):
            # persistent on-chip tensors (fp16)
            tt_res = pool_res.tile([P, EB, S], f16, tag="ttres")  # [d_in, d_out, s]
            v_res = pool_res.tile([P, SB, E], f16, tag="vres")  # [s_in, s_out, e]

            # PE warmup: dense dummy matmuls while the first input DMAs are
            # in flight, so HAM starts un-throttling before real work lands.
            # Parallel 3-engine DMA issue gets data on chip ~3us earlier than
            # serial sync-engine issue, so fewer warmups are needed.
            warm = pool_const.tile([P, 64], f16, tag="warm")
            nc.vector.memset(warm[:], 0.0)
            wps = pool_pr.tile([64, 64], f32, tag="pr", name="warm_ps")
            for i in range(NWARM):
                nc.tensor.matmul(
                    wps[:], lhsT=warm[:, :], rhs=warm[:, :],
                    start=True, stop=True,
                )

            # ---- Phase 1a: TT[d, s] = (k @ M^T)^T via weight WM = M^T ----
            wm_sb = None
            for ss in range(NPS):
                s_sl = slice(ss * PSTRIP, (ss + 1) * PSTRIP)
                if ss == 0:
                    # critical path: the first matmul needs kt chunk 0 plus
                    # wm chunk 0 -- issue those three transfers concurrently
                    # from three engines (a dma_start costs ~650ns of issue
                    # time on its engine), then stream the remaining chunk
                    # pairs in consumption order on sync (kt) + gpsimd (wm)
                    xin = [
                        pool_in0.tile(
                            [P, PSTRIP], f16, tag="in0", name=f"in0_{dc}"
                        )
                        for dc in range(DC)
                    ]
                    nc.sync.dma_start(xin[0][:], kt_v[:, 0, s_sl])
                    wm0a = pool_w.tile([P, 4 * P], f16, tag="w0a", name="w_m_0a")
                    wm0b = pool_w.tile([P, 4 * P], f16, tag="w0b", name="w_m_0b")
                    # both halves on gpsimd: the dc-major eb loop touches
                    # wm0a's columns ~0.85us before it needs wm0b's
                    nc.gpsimd.dma_start(wm0a[:], wm_v[:, 0, 0 : 4 * P])
                    nc.gpsimd.dma_start(wm0b[:], wm_v[:, 0, 4 * P : D])
                    # wm needs 2x the bytes of kt; queues get equal per-queue
                    # bandwidth, so spread wm over gpsimd+scalar (2 queues)
                    # to keep the weight stream ahead of consumption
                    wm_sb = [None]
                    for dc in range(1, DC):
                        nc.sync.dma_start(xin[dc][:], kt_v[:, dc, s_sl])
                        wt = pool_w.tile([P, D], f16, tag="w", name=f"w_m_{dc}")
                        # odd chunks on gpsimd (warm queue), evens on scalar
                        # (its queue starts ~1us late behind ACT_TABLE_LOAD)
                        eng = nc.gpsimd if dc % 2 else nc.scalar
                        eng.dma_start(wt[:], wm_v[:, dc, :])
                        wm_sb.append(wt)
                else:
                    xfull = pool_in.tile(
                        [P, DC, PSTRIP], f16, tag="in", name=f"in_k_{ss}"
                    )
                    nc.sync.dma_start(xfull[:], kt_v[:, :, s_sl])
                    xin = [xfull[:, dc, :] for dc in range(DC)]

                def wm_slice(dc, eb):
                    if dc == 0:
                        half = wm0a if eb < 4 else wm0b
                        return half[:, (eb % 4) * P : (eb % 4 + 1) * P]
                    return wm_sb[dc][:, eb * P : (eb + 1) * P]

                if ss == 0:
                    # dc-major over ALL 8 eb groups (7 po banks + the pr
                    # bank): matmul issue order matches weight-chunk DMA
                    # arrival, and the full-width pass cuts the required
                    # startup supply rate to ~225GB/s (vs 300 at NG=6),
                    # giving the DMA stream jitter margin so the PE
                    # doesn't stall on late chunks
                    grp = [
                        pool_po.tile(
                            [P, PSTRIP], f32, tag="po", name=f"ps_m0_{eb}"
                        )
                        for eb in range(EB - 1)
                    ] + [
                        pool_pr.tile(
                            [P, PSTRIP], f32, tag="pr", name="ps_m0_7"
                        )
                    ]
                    for dc in range(DC):
                        for eb in range(EB):
                            nc.tensor.matmul(
                                grp[eb][:],
                                lhsT=wm_slice(dc, eb),
                                rhs=xin[dc][:],
                                start=(dc == 0),
                                stop=(dc == DC - 1),
                            )
                    for eb in range(EB):
                        nc.scalar.activation(
                            tt_res[:, eb, s_sl], grp[eb][:], AF.Identity
                        )
                    eb_rest = range(0)
                else:
                    eb_rest = range(EB)
                for eb in eb_rest:
                    ps = pool_po.tile(
                        [P, PSTRIP], f32, tag="po", name=f"ps_m_{ss}_{eb}"
                    )
                    for dc in range(DC):
                        nc.tensor.matmul(
                            ps[:],
                            lhsT=wm_slice(dc, eb),
                            rhs=xin[dc][:],
                            start=(dc == 0),
                            stop=(dc == DC - 1),
                        )
                    nc.scalar.activation(tt_res[:, eb, s_sl], ps[:], AF.Identity)

            # constants deferred off the critical startup issue slots:
            # t3s is phase-2-only, bvb is phase-1b-only; ones is synthesized
            t3s_sb = pool_const.tile([P, SB], f32, tag="t3s")
            nc.sync.dma_start(t3s_sb[:], t3s[:])
            ones_sb = pool_const.tile([P, 1], f16, tag="ones")
            nc.gpsimd.memset(ones_sb[:], 1.0)
            bvb_sb = pool_const.tile([P, E], f32, tag="bvb")
            nc.sync.dma_start(bvb_sb[:], bvb[:])

            # ---- Phase 1b: V[s, e] = v @ Wv + bv ----
            wv_sb = None
            for ss in range(NPS):
                vin = pool_in.tile(
                    [P, DC, PSTRIP], f16, tag="in", name=f"in_v_{ss}"
                )
                nc.sync.dma_start(
                    vin[:], vt_v[:, :, ss * PSTRIP : (ss + 1) * PSTRIP]
                )
                if wv_sb is None:
                    wv0 = pool_w.tile([P, E], f16, tag="w", name="w_v_0")
                    nc.sync.dma_start(wv0[:], wv_v[:, 0, :])
                    wvr = pool_wr.tile(
                        [P, DC - 1, E], f16, tag="wr", name="w_v_r"
                    )
                    nc.sync.dma_start(wvr[:], wv_v[:, 1:, :])
                    wv_sb = [wv0[:, :]] + [
                        wvr[:, dc - 1, :] for dc in range(1, DC)
                    ]
                for sbl in range(PSTRIP // P):  # s blocks within strip
                    sb = ss * (PSTRIP // P) + sbl
                    for es in range(NES):
                        e_sl = slice(es * ESTRIP, (es + 1) * ESTRIP)
                        ps = pool_po.tile(
                            [P, ESTRIP], f32, tag="po", name=f"ps_v_{ss}_{sbl}_{es}"
                        )
                        for dc in range(DC):
                            nc.tensor.matmul(
                                ps[:],
                                lhsT=vin[:, dc, sbl * P : (sbl + 1) * P],
                                rhs=wv_sb[dc][:, e_sl],
                                start=(dc == 0),
                                stop=(dc == DC - 1),
                            )
                        nc.vector.tensor_add(
                            v_res[:, sb, e_sl], ps[:], bvb_sb[:, e_sl]
                        )

            # ---- Phase 2: attention, per 512-wide q strip, two passes ----
            for qs in range(NQS):
                q_sl = slice(qs * QSTRIP, (qs + 1) * QSTRIP)
                qin = pool_in.tile(
                    [P, DC, QSTRIP], f16, tag="in", name=f"in_q_{qs}"
                )
                nc.sync.dma_start(qin[:], qt_v[:, :, q_sl])

                # pass A: all 16 attnT tiles + running fp32 column sums
                at_tiles = []
                acc = pool_acc.tile([P, QSTRIP], f32, tag="acc", name=f"acc_{qs}")
                for kb in range(SB):
                    ps = pool_po.tile(
                        [P, QSTRIP], f32, tag="po", name=f"ps_s_{qs}_{kb}"
                    )
                    for ec in range(EB):
                        nc.tensor.matmul(
                            ps[:],
                            lhsT=tt_res[:, ec, kb * P : (kb + 1) * P],
                            rhs=qin[:, ec, :],
                            start=(ec == 0),
                            stop=(ec == EB - 1),
                        )
                    at = pool_at.tile(
                        [P, QSTRIP], f16, tag="at", name=f"at_{qs}_{kb}"
                    )
                    nc.scalar.activation(
                        at[:], ps[:], AF.Exp, scale=SCALE,
                        bias=t3s_sb[:, kb : kb + 1],
                    )
                    if kb == 0:
                        nc.vector.tensor_copy(acc[:], at[:])
                    else:
                        nc.vector.tensor_add(acc[:], acc[:], at[:])
                    at_tiles.append(at)

                # row sums -> reciprocal, via ones matmul on fp16 copy of acc
                acc16 = pool_acc.tile(
                    [P, QSTRIP], f16, tag="acc16", name=f"acc16_{qs}"
                )
                nc.vector.tensor_copy(acc16[:], acc[:])
                pr = pool_pr.tile([P, NQB], f32, tag="pr", name=f"pr_{qs}")
                for qb in range(NQB):
                    nc.tensor.matmul(
                        pr[:, qb : qb + 1],
                        lhsT=acc16[:, qb * P : (qb + 1) * P],
                        rhs=ones_sb[:],
                        start=True,
                        stop=True,
                    )
                recip = pool_small.tile(
                    [P, NQB], f32, tag="recip", name=f"recip_{qs}"
                )
                nc.vector.reciprocal(recip[:], pr[:])

                # pass B: attnT.T @ V, one e-half at a time (all N=512)
                for es in range(NES):
                    e_sl = slice(es * ESTRIP, (es + 1) * ESTRIP)
                    out_ps = [
                        pool_po.tile(
                            [P, ESTRIP], f32, tag="po", name=f"ops_{qs}_{es}_{qb}"
                        )
                        for qb in range(NQB)
                    ]
                    # qb-outer: each q block's accumulation finishes early so
                    # its normalize+store overlaps the remaining matmuls
                    for qb in range(NQB):
                        last_block = (
                            qs == NQS - 1 and es == NES - 1 and qb == NQB - 1
                        )
                        if last_block:
                            # final block: two 256-wide chains, normalize on
                            # the (idle) scalar engine -- half the store +
                            # normalize latency comes off the kernel tail
                            EH = ESTRIP // 2
                            for hf in range(2):
                                h_sl = slice(
                                    es * ESTRIP + hf * EH,
                                    es * ESTRIP + (hf + 1) * EH,
                                )
                                psh = pool_po.tile(
                                    [P, EH], f32, tag="po", name=f"ops_fin_{hf}"
                                )
                                for kb in range(SB):
                                    nc.tensor.matmul(
                                        psh[:],
                                        lhsT=at_tiles[kb][:, qb * P : (qb + 1) * P],
                                        rhs=v_res[:, kb, h_sl],
                                        start=(kb == 0),
                                        stop=(kb == SB - 1),
                                    )
                                sth = pool_stage.tile(
                                    [P, EH], f16, tag="ost2", name=f"osth_{hf}"
                                )
                                nc.scalar.activation(
                                    sth[:], psh[:], AF.Copy,
                                    scale=recip[:, qb : qb + 1],
                                )
                                nc.sync.dma_start(
                                    out[
                                        qs * QSTRIP + qb * P : qs * QSTRIP
                                        + (qb + 1) * P,
                                        h_sl,
                                    ],
                                    sth[:],
                                )
                            continue
                        for kb in range(SB):
                            nc.tensor.matmul(
                                out_ps[qb][:],
                                lhsT=at_tiles[kb][:, qb * P : (qb + 1) * P],
                                rhs=v_res[:, kb, e_sl],
                                start=(kb == 0),
                                stop=(kb == SB - 1),
                            )
                        st = pool_stage.tile(
                            [P, ESTRIP], f16, tag="ostage", name=f"ost_{qs}_{es}_{qb}"
                        )
                        nc.vector.tensor_scalar_mul(
                            st[:], out_ps[qb][:], recip[:, qb : qb + 1]
                        )
                        nc.sync.dma_start(
                            out[
                                qs * QSTRIP + qb * P : qs * QSTRIP + (qb + 1) * P,
                                e_sl,
                            ],
                            st[:],
                        )

    nc.compile()
    return nc


def _get_nc():
    if "nc" not in _CACHE:
        _CACHE["nc"] = _build()
    return _CACHE["nc"]


def run(inputs, trace=False):
    from concourse.bass_utils import run_bass_kernel_spmd

    nc = _get_nc()
    n_cores = 8
    f = np.float32
    h = np.float16

    query = np.asarray(inputs["query"])
    key = np.asarray(inputs["key"])
    value = np.asarray(inputs["value"])
    Wq = np.asarray(inputs["Wq"], f)
    Wk = np.asarray(inputs["Wk"], f)
    Wv = np.asarray(inputs["Wv"], f)
    bq = np.asarray(inputs["bq"], f)
    bv = np.asarray(inputs["bv"], f)

    # host-side folding (shared across cores)
    wm = np.ascontiguousarray((Wk @ Wq.T).astype(h))  # [d_k, d_q]
    w3 = Wk @ bq  # t3 = k @ w3, per-k bias (scaled below)
    wv16 = np.ascontiguousarray(Wv.astype(h))
    bvb = np.ascontiguousarray(np.broadcast_to(bv, (P, E)))

    in_maps = []
    for b in range(n_cores):
        kb32 = np.asarray(key[b], f)
        t3 = (kb32 @ w3) * np.float32(SCALE)  # [S]
        in_maps.append({
            "qt": np.ascontiguousarray(np.asarray(query[b]).T.astype(h)),
            "kt": np.ascontiguousarray(kb32.T.astype(h)),
            "vt": np.ascontiguousarray(np.asarray(value[b]).T.astype(h)),
            "wm": wm,
            "wv": wv16,
            "t3s": np.ascontiguousarray(t3.reshape(SB, P).T.astype(f)),
            "bvb": bvb,
        })

    # the axon-tunneled device occasionally wedges transiently
    # (NRT_EXEC_UNIT_UNRECOVERABLE) and recovers on re-execution
    last = None
    for attempt in range(3):
        try:
            res = run_bass_kernel_spmd(
                nc, in_maps, core_ids=list(range(n_cores)), trace=trace
            )
            break
        except Exception as e:
            last = e
            import time as _time

            _time.sleep(5.0)
    else:
        raise last
    out = np.stack([r["out"] for r in res.results], axis=0)
    return out.astype(np.float32), res


def kernel(**inputs):
    return run(inputs, trace=False)[0]

